# revision 28
# baseline (speedup 1.0000x reference)
"""Trainium2 Bass kernel for nn_Attention_Dec (dense cross-attention, B=2 N=2048
DIM=1024 H=16 heads of d=64, padding mask, softmax over x-positions).

Sharding: core c handles batch b=c//4 and 4 heads h0=(c%4)*4 (tensor-parallel
over heads within a batch).  Each core computes a partial output
Y_c = (softmax-attn for its 4 heads) @ W_out[:, cols].T  of shape [2048, 1024];
host sums the 4 partials per batch, adds b_out, and writes NaN rows where the
(front-padded) mask is False — matching the reference's all--inf softmax NaNs.

Device-side per core (all matmuls float32r, fp32 PSUM accumulate):
  QT[d,j] = (Wq_h/32) @ x^T       (j = x positions; scale folded into weights)
  V[j,d]  = x @ Wv_h^T            (+ ones column for the softmax denominator)
  KT[d,i] = Wk_h @ tar^T          (i = tar positions)
  S^T[j,i] = QT^T·KT  (per head, K=64 contraction, head pairs row-tiled)
  P^T = exp(S^T + maskadd_j)      (per-partition ACT bias masks x positions;
                                   -30000 underflows exp to exactly 0)
  OT[d,i] = sum_j V_aug[j,d]·P^T[j,i]  (row 64 = softmax denominator den_i)
  AT[d,i] = OT[d,i] * (1/den_i)   (gpsimd partition_broadcast of the den row +
                                   DVE reciprocal_approx_fast + tensor_mul)
  Y[n,o] += AT_pair[:,n]^T @ WoT_pair  (K=128, accumulated over head pairs)

PSUM: 4 two-bank slots (sA, sB, otA, otB).  Attention keeps S^T ping-ponging
on sA/sB while otA/otB hold the two PV accumulators of the current head pair;
projections/final reuse the same slots.
"""

from contextlib import ExitStack

import numpy as np

B, N, DIM, H = 2, 2048, 1024, 16
D = 64
HPC = 4  # heads per core
NCORES = 8
NJC = N // 128  # 16 j-chunks
NKC = DIM // 128  # 8 contraction chunks

_CACHE = {}


def _build_nc():
    import concourse.tile as tile
    from concourse import bacc, mybir
    from concourse.tile import add_dep_helper

    F32 = mybir.dt.float32
    F32R = mybir.dt.float32r
    EXP = mybir.ActivationFunctionType.Exp

    nc = bacc.Bacc("TRN2", debug=False, num_devices=NCORES)

    xT = nc.dram_tensor("xT", [DIM, N], F32R, kind="ExternalInput").ap()
    tarT = nc.dram_tensor("tarT", [DIM, N], F32R, kind="ExternalInput").ap()
    wqT = nc.dram_tensor("wqT", [DIM, HPC * D], F32R, kind="ExternalInput").ap()
    wvT = nc.dram_tensor("wvT", [DIM, HPC * D], F32R, kind="ExternalInput").ap()
    wkT = nc.dram_tensor("wkT", [DIM, HPC * D], F32R, kind="ExternalInput").ap()
    woTp = nc.dram_tensor("woTp", [2, 128, DIM], F32R, kind="ExternalInput").ap()
    maskadd = nc.dram_tensor("maskadd", [128, NJC], F32, kind="ExternalInput").ap()
    onesc = nc.dram_tensor("onesc", [128, HPC], F32R, kind="ExternalInput").ap()
    y = nc.dram_tensor("y", [N, DIM], F32, kind="ExternalOutput").ap()

    with tile.TileContext(nc) as tc, ExitStack() as ctx:
        consts = ctx.enter_context(tc.tile_pool(name="consts", bufs=1))
        wpool = ctx.enter_context(tc.tile_pool(name="wpool", bufs=1))
        qkv = ctx.enter_context(tc.tile_pool(name="qkv", bufs=1))
        ps2 = ctx.enter_context(tc.tile_pool(name="ps2", bufs=1, space="PSUM"))
        pot = ctx.enter_context(tc.tile_pool(name="pot", bufs=1, space="PSUM"))

        mk = consts.tile([128, NJC], F32, name="mk")
        nc.scalar.dma_start(mk[:], maskadd[:])
        ones_c = consts.tile([128, HPC], F32R, name="ones_c")
        nc.scalar.dma_start(ones_c[:], onesc[:])

        wq_t, wv_t, wk_t = [], [], []

        KT = [qkv.tile([128, N], F32R, name=f"KT{p}", tag=f"KT{p}") for p in range(2)]
        QT = [qkv.tile([128, N], F32R, name=f"QT{p}", tag=f"QT{p}") for p in range(2)]
        VA_all = qkv.tile([128, NJC, HPC, D + 1], F32R, name="VA_all")
        VA = [VA_all[:, jc] for jc in range(NJC)]
        PTAGS = ["sA", "sB", "otA", "otB"]

        def proj_psum(i, nm):
            return (ps2 if i < 2 else pot).tile(
                [128, N // 2], F32, name=f"{nm}{i}", tag=PTAGS[i], bufs=1
            )

        ptp = ctx.enter_context(tc.tile_pool(name="ptp", bufs=10))

        def att_S(p, ih, jc):
            """S^T pair matmuls + masked exp for one j-chunk; returns PT pair."""
            ioff = ih * 1024
            stA = ps2.tile(
                [128, N // 2], F32, name=f"stA{p}_{jc}_{ih}", tag="sA", bufs=1
            )
            stB = ps2.tile(
                [128, N // 2], F32, name=f"stB{p}_{jc}_{ih}", tag="sB", bufs=1
            )
            for ib in range(2):
                nc.tensor.matmul(
                    stA[:, ib * 512 : (ib + 1) * 512],
                    QT[p][0:D, jc * 128 : (jc + 1) * 128],
                    KT[p][0:D, ioff + ib * 512 : ioff + (ib + 1) * 512],
                    start=True,
                    stop=True,
                )
            for ib in range(2):
                nc.tensor.matmul(
                    stB[:, ib * 512 : (ib + 1) * 512],
                    QT[p][D:128, jc * 128 : (jc + 1) * 128],
                    KT[p][D:128, ioff + ib * 512 : ioff + (ib + 1) * 512],
                    start=True,
                    stop=True,
                )
            ptA = ptp.tile([128, N // 2], F32R, name=f"ptA{p}_{jc}_{ih}", tag="pt")
            nc.scalar.activation(
                ptA[:], stA[:], EXP, bias=mk[:, jc : jc + 1], scale=1.0
            )
            ptB = ptp.tile([128, N // 2], F32R, name=f"ptB{p}_{jc}_{ih}", tag="pt")
            nc.scalar.activation(
                ptB[:], stB[:], EXP, bias=mk[:, jc : jc + 1], scale=1.0
            )
            return (jc, ptA, ptB)

        # ---- QT + V from xT ----
        with tc.tile_pool(name="xt", bufs=1) as xtp:
            xt = []
            for kc in range(NKC):
                t = xtp.tile([128, N], F32R, name=f"xt{kc}", tag=f"xt{kc}")
                [nc.sync, nc.scalar, nc.gpsimd][kc % 3].dma_start(
                    t[:], xT[kc * 128 : (kc + 1) * 128, :]
                )
                xt.append(t)
                # weights ride between the activation chunks (xt0 lands first)
                t = wpool.tile([128, HPC * D], F32R, name=f"wq{kc}", tag=f"wq{kc}")
                nc.sync.dma_start(t[:], wqT[kc * 128 : (kc + 1) * 128, :])
                wq_t.append(t)
                t = wpool.tile([128, HPC * D], F32R, name=f"wv{kc}", tag=f"wv{kc}")
                nc.scalar.dma_start(t[:], wvT[kc * 128 : (kc + 1) * 128, :])
                wv_t.append(t)
            # KT weights + tarT queue up behind xT on the DMA ring
            for kc in range(NKC):
                t = wpool.tile([128, HPC * D], F32R, name=f"wk{kc}", tag=f"wk{kc}")
                nc.scalar.dma_start(t[:], wkT[kc * 128 : (kc + 1) * 128, :])
                wk_t.append(t)

            psq = [proj_psum(i, "psq") for i in range(4)]
            for kc in range(NKC):
                for p in range(2):
                    for hf in range(2):
                        for ib in range(2):
                            off = hf * 1024 + ib * 512
                            nc.tensor.matmul(
                                psq[2 * p + hf][:, ib * 512 : (ib + 1) * 512],
                                wq_t[kc][:, p * 128 : (p + 1) * 128],
                                xt[kc][:, off : off + 512],
                                start=(kc == 0),
                                stop=(kc == NKC - 1),
                            )
            for p in range(2):
                for hf in range(2):
                    nc.vector.tensor_copy(
                        QT[p][:, hf * 1024 : (hf + 1) * 1024], psq[2 * p + hf][:]
                    )
            for jc in range(NJC):
                pv = (ps2 if jc % 2 == 0 else pot).tile(
                    [128, HPC * D], F32, name=f"pv{jc}",
                    tag=PTAGS[(jc % 2) * 2], bufs=1,
                )
                for kc in range(NKC):
                    nc.tensor.matmul(
                        pv[:],
                        xt[kc][:, jc * 128 : (jc + 1) * 128],
                        wv_t[kc][:],
                        start=(kc == 0),
                        stop=(kc == NKC - 1),
                    )
                nc.vector.tensor_copy(VA[jc][:, :, 0:D], pv[:])
                nc.vector.tensor_copy(VA[jc][:, :, D], ones_c[:])

        # ---- KT from tarT ----
        with tc.tile_pool(name="tart", bufs=1) as tartp:
            tt = []
            for kc in range(NKC):
                t = tartp.tile([128, N], F32R, name=f"tart{kc}", tag=f"tart{kc}")
                [nc.sync, nc.scalar, nc.gpsimd][kc % 3].dma_start(
                    t[:], tarT[kc * 128 : (kc + 1) * 128, :]
                )
                tt.append(t)
            psk = [proj_psum(i, "psk") for i in range(4)]
            for kc in range(NKC):
                for hf in range(2):
                    for ib in range(2):
                        off = hf * 1024 + ib * 512
                        nc.tensor.matmul(
                            psk[hf][:, ib * 512 : (ib + 1) * 512],
                            wk_t[kc][:, 0:128],
                            tt[kc][:, off : off + 512],
                            start=(kc == 0),
                            stop=(kc == NKC - 1),
                        )
            for hf in range(2):
                nc.vector.tensor_copy(
                    KT[0][:, hf * 1024 : (hf + 1) * 1024], psk[hf][:]
                )
            # attention pipeline prefill: S+exp for (p0,ih0) jc0; the p1 KT
            # matmuls below keep the PE dense while the first exps run
            prefill = [att_S(0, 0, 0)]
            for kc in range(NKC):
                for hf in range(2):
                    for ib in range(2):
                        off = hf * 1024 + ib * 512
                        nc.tensor.matmul(
                            psk[2 + hf][:, ib * 512 : (ib + 1) * 512],
                            wk_t[kc][:, 128:256],
                            tt[kc][:, off : off + 512],
                            start=(kc == 0),
                            stop=(kc == NKC - 1),
                        )
            for hf in range(2):
                nc.vector.tensor_copy(
                    KT[1][:, hf * 1024 : (hf + 1) * 1024], psk[2 + hf][:]
                )
            prefill.append(att_S(0, 0, 1))

        # ---- late pool (reuses released xt/tart space) ----
        late = ctx.enter_context(tc.tile_pool(name="late", bufs=1))
        wo_t = []
        for p in range(2):
            t = late.tile([128, DIM], F32R, name=f"wo{p}", tag=f"wo{p}")
            nc.sync.dma_start(t[:], woTp[p])
            wo_t.append(t)
        OTs = [
            late.tile([D + 1, N], F32, name=f"OTs{h}", tag=f"OTs{h}")
            for h in range(HPC)
        ]
        AT = [late.tile([128, N], F32R, name=f"AT{p}", tag=f"AT{p}") for p in range(2)]
        rbd = ctx.enter_context(tc.tile_pool(name="rbd", bufs=2))
        ysb = ctx.enter_context(tc.tile_pool(name="ysb", bufs=3))

        def norm_head(h, ih):
            # AT[pair] rows <- OTs_h[0:D] * broadcast(1/den_h), one i-half
            p, lo = h // 2, (h % 2) * D
            io = ih * 1024
            dtmp = rbd.tile([1, N // 2], F32, name=f"dtmp{h}_{ih}", tag="dtmp")
            nc.vector.tensor_copy(dtmp[:], OTs[h][D : D + 1, io : io + 1024])
            bden = rbd.tile([D, N // 2], F32, name=f"bden{h}_{ih}", tag="bden")
            nc.gpsimd.partition_broadcast(bden[:], dtmp[0:1, :], channels=D)
            rbb = rbd.tile([D, N // 2], F32, name=f"rbb{h}_{ih}", tag="rbb")
            nc.vector.reciprocal_approx_fast(rbb[:], bden[:])
            nc.vector.tensor_mul(
                AT[p][lo : lo + D, io : io + 1024],
                OTs[h][0:D, io : io + 1024],
                rbb[:],
            )

        # ---- attention ----
        for p in range(2):
            hA, hB = 2 * p, 2 * p + 1
            for ih in range(2):
                ioff = ih * 1024
                otA = pot.tile([D + 1, N // 2], F32, name=f"otA{p}_{ih}", tag="otA")
                otB = pot.tile([D + 1, N // 2], F32, name=f"otB{p}_{ih}", tag="otB")
                if p == 0 and ih == 0:
                    pending = list(prefill)
                else:
                    pending = []
                for jc in range(len(pending), NJC):
                    pending.append(att_S(p, ih, jc))
                    # flush PV matmuls in 16-mm bursts (keeps PE runs long)
                    if jc % 4 == 3:
                        for jcf, pA, pB in pending:
                            for ib in range(2):
                                nc.tensor.matmul(
                                    otA[:, ib * 512 : (ib + 1) * 512],
                                    VA[jcf][:, hA, :],
                                    pA[:, ib * 512 : (ib + 1) * 512],
                                    start=(jcf == 0),
                                    stop=(jcf == NJC - 1),
                                )
                            for ib in range(2):
                                nc.tensor.matmul(
                                    otB[:, ib * 512 : (ib + 1) * 512],
                                    VA[jcf][:, hB, :],
                                    pB[:, ib * 512 : (ib + 1) * 512],
                                    start=(jcf == 0),
                                    stop=(jcf == NJC - 1),
                                )
                        pending = []
                nc.vector.tensor_copy(OTs[hA][:, ioff : ioff + 1024], otA[:])
                nc.vector.tensor_copy(OTs[hB][:, ioff : ioff + 1024], otB[:])
                # normalize this i-half while the next block's attention runs
                norm_head(hA, ih)
                norm_head(hB, ih)

        # ---- output projection: Y[n,:] = sum_p AT_p[:,n]^T @ WoT_p (K=128) ----
        for nchunk in range(NJC):
            py = (ps2 if nchunk % 4 < 2 else pot).tile(
                [128, DIM], F32, name=f"py{nchunk}", tag=PTAGS[nchunk % 4],
                bufs=1,
            )
            for oh in range(2):
                for p in range(2):
                    nc.tensor.matmul(
                        py[:, oh * 512 : (oh + 1) * 512],
                        AT[p][:, nchunk * 128 : (nchunk + 1) * 128],
                        wo_t[p][:, oh * 512 : (oh + 1) * 512],
                        start=(p == 0),
                        stop=(p == 1),
                    )
            yt = ysb.tile([128, DIM], F32, name=f"yt{nchunk}", tag="yt")
            if nchunk % 2 == 0:
                nc.scalar.copy(yt[:], py[:])
            else:
                nc.vector.tensor_copy(yt[:], py[:])
            (nc.sync if nchunk % 2 == 0 else nc.scalar).dma_start(
                y[nchunk * 128 : (nchunk + 1) * 128, :], yt[:]
            )

    nc.compile()
    return nc


def _get_nc():
    if "nc" not in _CACHE:
        _CACHE["nc"] = _build_nc()
    return _CACHE["nc"]


def kernel(x, tar, mask, W_qv, W_k, W_out, b_out):
    from concourse import bass_utils

    x = np.asarray(x, np.float32)
    tar = np.asarray(tar, np.float32)
    mask = np.asarray(mask).astype(bool)
    W_qv = np.asarray(W_qv, np.float32)
    W_k = np.asarray(W_k, np.float32)
    W_out = np.asarray(W_out, np.float32)
    b_out = np.asarray(b_out, np.float32)

    m_pad = np.concatenate([np.ones((B, 1), bool), mask], axis=1)  # [B, N]
    maskadd_f = np.where(m_pad, 0.0, -30000.0).astype(np.float32)

    nc = _get_nc()
    in_maps = []
    for c in range(NCORES):
        b = c // 4
        h0 = (c % 4) * HPC
        rows = slice(h0 * D, h0 * D + HPC * D)
        vrows = slice(DIM + h0 * D, DIM + h0 * D + HPC * D)
        in_maps.append(
            {
                "xT": np.ascontiguousarray(x[b].T),
                "tarT": np.ascontiguousarray(tar[b].T),
                "wqT": np.ascontiguousarray((W_qv[rows] * np.float32(0.03125)).T),
                "wvT": np.ascontiguousarray(W_qv[vrows].T),
                "wkT": np.ascontiguousarray(W_k[rows].T),
                "woTp": np.ascontiguousarray(W_out[:, rows].T.reshape(2, 128, DIM)),
                "maskadd": np.ascontiguousarray(maskadd_f[b].reshape(NJC, 128).T),
                "onesc": np.ones((128, HPC), np.float32),
            }
        )

    res = bass_utils.run_bass_kernel_spmd(nc, in_maps, core_ids=list(range(NCORES)))
    out = np.empty((B, N, DIM), np.float32)
    for b in range(B):
        acc = res.results[4 * b]["y"].copy()
        for c in range(4 * b + 1, 4 * b + 4):
            acc += res.results[c]["y"]
        acc += b_out
        acc[~m_pad[b]] = np.nan
        out[b] = acc
    return out


# revision 29
# speedup vs baseline: 1.0090x; 1.0090x over previous
"""Trainium2 Bass kernel for nn_Attention_Dec (dense cross-attention, B=2 N=2048
DIM=1024 H=16 heads of d=64, padding mask, softmax over x-positions).

Sharding: core c handles batch b=c//4 and 4 heads h0=(c%4)*4 (tensor-parallel
over heads within a batch).  Each core computes a partial output
Y_c = (softmax-attn for its 4 heads) @ W_out[:, cols].T  of shape [2048, 1024];
host sums the 4 partials per batch, adds b_out, and writes NaN rows where the
(front-padded) mask is False — matching the reference's all--inf softmax NaNs.

Device-side per core (all matmuls float32r, fp32 PSUM accumulate):
  QT[d,j] = (Wq_h/32) @ x^T       (j = x positions; scale folded into weights)
  V[j,d]  = x @ Wv_h^T            (+ ones column for the softmax denominator)
  KT[d,i] = Wk_h @ tar^T          (i = tar positions)
  S^T[j,i] = QT^T·KT  (per head, K=64 contraction, head pairs row-tiled)
  P^T = exp(S^T + maskadd_j)      (per-partition ACT bias masks x positions;
                                   -30000 underflows exp to exactly 0)
  OT[d,i] = sum_j V_aug[j,d]·P^T[j,i]  (row 64 = softmax denominator den_i)
  AT[d,i] = OT[d,i] * (1/den_i)   (gpsimd partition_broadcast of the den row +
                                   DVE reciprocal_approx_fast + tensor_mul)
  Y[n,o] += AT_pair[:,n]^T @ WoT_pair  (K=128, accumulated over head pairs)

PSUM: 4 two-bank slots (sA, sB, otA, otB).  Attention keeps S^T ping-ponging
on sA/sB while otA/otB hold the two PV accumulators of the current head pair;
projections/final reuse the same slots.
"""

from contextlib import ExitStack

import numpy as np

B, N, DIM, H = 2, 2048, 1024, 16
D = 64
HPC = 4  # heads per core
NCORES = 8
NJC = N // 128  # 16 j-chunks
NKC = DIM // 128  # 8 contraction chunks

_CACHE = {}


def _build_nc():
    import concourse.tile as tile
    from concourse import bacc, mybir
    from concourse.tile import add_dep_helper

    F32 = mybir.dt.float32
    F32R = mybir.dt.float32r
    EXP = mybir.ActivationFunctionType.Exp

    nc = bacc.Bacc("TRN2", debug=False, num_devices=NCORES)

    xT = nc.dram_tensor("xT", [DIM, N], F32R, kind="ExternalInput").ap()
    tarT = nc.dram_tensor("tarT", [DIM, N], F32R, kind="ExternalInput").ap()
    wqT = nc.dram_tensor("wqT", [DIM, HPC * D], F32R, kind="ExternalInput").ap()
    wvT = nc.dram_tensor("wvT", [DIM, HPC * D], F32R, kind="ExternalInput").ap()
    wkT = nc.dram_tensor("wkT", [DIM, HPC * D], F32R, kind="ExternalInput").ap()
    woTp = nc.dram_tensor("woTp", [2, 128, DIM], F32R, kind="ExternalInput").ap()
    maskadd = nc.dram_tensor("maskadd", [128, NJC], F32, kind="ExternalInput").ap()
    onesc = nc.dram_tensor("onesc", [128, HPC], F32R, kind="ExternalInput").ap()
    y = nc.dram_tensor("y", [N, DIM], F32, kind="ExternalOutput").ap()

    with tile.TileContext(nc) as tc, ExitStack() as ctx:
        consts = ctx.enter_context(tc.tile_pool(name="consts", bufs=1))
        wpool = ctx.enter_context(tc.tile_pool(name="wpool", bufs=1))
        qkv = ctx.enter_context(tc.tile_pool(name="qkv", bufs=1))
        ps2 = ctx.enter_context(tc.tile_pool(name="ps2", bufs=1, space="PSUM"))
        pot = ctx.enter_context(tc.tile_pool(name="pot", bufs=1, space="PSUM"))

        mk = consts.tile([128, NJC], F32, name="mk")
        nc.scalar.dma_start(mk[:], maskadd[:])
        ones_c = consts.tile([128, HPC], F32R, name="ones_c")
        nc.scalar.dma_start(ones_c[:], onesc[:])

        wq_t, wv_t, wk_t = [], [], []

        KT = [qkv.tile([128, N], F32R, name=f"KT{p}", tag=f"KT{p}") for p in range(2)]
        QT = [qkv.tile([128, N], F32R, name=f"QT{p}", tag=f"QT{p}") for p in range(2)]
        VA_all = qkv.tile([128, NJC, HPC, D + 1], F32R, name="VA_all")
        VA = [VA_all[:, jc] for jc in range(NJC)]
        PTAGS = ["sA", "sB", "otA", "otB"]

        def proj_psum(i, nm):
            return (ps2 if i < 2 else pot).tile(
                [128, N // 2], F32, name=f"{nm}{i}", tag=PTAGS[i], bufs=1
            )

        ptp = ctx.enter_context(tc.tile_pool(name="ptp", bufs=10))

        def att_S(p, ih, jc):
            """S^T pair matmuls + masked exp for one j-chunk; returns PT pair."""
            ioff = ih * 1024
            stA = ps2.tile(
                [128, N // 2], F32, name=f"stA{p}_{jc}_{ih}", tag="sA", bufs=1
            )
            stB = ps2.tile(
                [128, N // 2], F32, name=f"stB{p}_{jc}_{ih}", tag="sB", bufs=1
            )
            for ib in range(2):
                nc.tensor.matmul(
                    stA[:, ib * 512 : (ib + 1) * 512],
                    QT[p][0:D, jc * 128 : (jc + 1) * 128],
                    KT[p][0:D, ioff + ib * 512 : ioff + (ib + 1) * 512],
                    start=True,
                    stop=True,
                )
            for ib in range(2):
                nc.tensor.matmul(
                    stB[:, ib * 512 : (ib + 1) * 512],
                    QT[p][D:128, jc * 128 : (jc + 1) * 128],
                    KT[p][D:128, ioff + ib * 512 : ioff + (ib + 1) * 512],
                    start=True,
                    stop=True,
                )
            ptA = ptp.tile([128, N // 2], F32R, name=f"ptA{p}_{jc}_{ih}", tag="pt")
            nc.scalar.activation(
                ptA[:], stA[:], EXP, bias=mk[:, jc : jc + 1], scale=1.0
            )
            ptB = ptp.tile([128, N // 2], F32R, name=f"ptB{p}_{jc}_{ih}", tag="pt")
            nc.scalar.activation(
                ptB[:], stB[:], EXP, bias=mk[:, jc : jc + 1], scale=1.0
            )
            return (jc, ptA, ptB)

        # ---- QT + V from xT ----
        with tc.tile_pool(name="xt", bufs=1) as xtp:
            xt = []
            for kc in range(NKC):
                t = xtp.tile([128, N], F32R, name=f"xt{kc}", tag=f"xt{kc}")
                (nc.sync if kc % 2 == 0 else nc.scalar).dma_start(
                    t[:], xT[kc * 128 : (kc + 1) * 128, :]
                )
                xt.append(t)
                # weights ride between the activation chunks (xt0 lands first)
                t = wpool.tile([128, HPC * D], F32R, name=f"wq{kc}", tag=f"wq{kc}")
                nc.sync.dma_start(t[:], wqT[kc * 128 : (kc + 1) * 128, :])
                wq_t.append(t)
                t = wpool.tile([128, HPC * D], F32R, name=f"wv{kc}", tag=f"wv{kc}")
                nc.scalar.dma_start(t[:], wvT[kc * 128 : (kc + 1) * 128, :])
                wv_t.append(t)
            # KT weights + tarT queue up behind xT on the DMA ring
            for kc in range(NKC):
                t = wpool.tile([128, HPC * D], F32R, name=f"wk{kc}", tag=f"wk{kc}")
                nc.scalar.dma_start(t[:], wkT[kc * 128 : (kc + 1) * 128, :])
                wk_t.append(t)

            psq = [proj_psum(i, "psq") for i in range(4)]
            for kc in range(NKC):
                for p in range(2):
                    for hf in range(2):
                        for ib in range(2):
                            off = hf * 1024 + ib * 512
                            nc.tensor.matmul(
                                psq[2 * p + hf][:, ib * 512 : (ib + 1) * 512],
                                wq_t[kc][:, p * 128 : (p + 1) * 128],
                                xt[kc][:, off : off + 512],
                                start=(kc == 0),
                                stop=(kc == NKC - 1),
                            )
            for p in range(2):
                for hf in range(2):
                    nc.vector.tensor_copy(
                        QT[p][:, hf * 1024 : (hf + 1) * 1024], psq[2 * p + hf][:]
                    )
            for jc in range(NJC):
                pv = (ps2 if jc % 2 == 0 else pot).tile(
                    [128, HPC * D], F32, name=f"pv{jc}",
                    tag=PTAGS[(jc % 2) * 2], bufs=1,
                )
                for kc in range(NKC):
                    nc.tensor.matmul(
                        pv[:],
                        xt[kc][:, jc * 128 : (jc + 1) * 128],
                        wv_t[kc][:],
                        start=(kc == 0),
                        stop=(kc == NKC - 1),
                    )
                nc.vector.tensor_copy(VA[jc][:, :, 0:D], pv[:])
                nc.vector.tensor_copy(VA[jc][:, :, D], ones_c[:])

        # ---- KT from tarT ----
        with tc.tile_pool(name="tart", bufs=1) as tartp:
            tt = []
            for kc in range(NKC):
                t = tartp.tile([128, N], F32R, name=f"tart{kc}", tag=f"tart{kc}")
                (nc.sync if kc % 2 == 0 else nc.scalar).dma_start(
                    t[:], tarT[kc * 128 : (kc + 1) * 128, :]
                )
                tt.append(t)
            psk = [proj_psum(i, "psk") for i in range(4)]
            for kc in range(NKC):
                for hf in range(2):
                    for ib in range(2):
                        off = hf * 1024 + ib * 512
                        nc.tensor.matmul(
                            psk[hf][:, ib * 512 : (ib + 1) * 512],
                            wk_t[kc][:, 0:128],
                            tt[kc][:, off : off + 512],
                            start=(kc == 0),
                            stop=(kc == NKC - 1),
                        )
            for hf in range(2):
                nc.vector.tensor_copy(
                    KT[0][:, hf * 1024 : (hf + 1) * 1024], psk[hf][:]
                )
            # attention pipeline prefill: S+exp for (p0,ih0) jc0; the p1 KT
            # matmuls below keep the PE dense while the first exps run
            prefill = [att_S(0, 0, 0)]
            for kc in range(NKC):
                for hf in range(2):
                    for ib in range(2):
                        off = hf * 1024 + ib * 512
                        nc.tensor.matmul(
                            psk[2 + hf][:, ib * 512 : (ib + 1) * 512],
                            wk_t[kc][:, 128:256],
                            tt[kc][:, off : off + 512],
                            start=(kc == 0),
                            stop=(kc == NKC - 1),
                        )
            for hf in range(2):
                nc.vector.tensor_copy(
                    KT[1][:, hf * 1024 : (hf + 1) * 1024], psk[2 + hf][:]
                )
            prefill.append(att_S(0, 0, 1))

        # ---- late pool (reuses released xt/tart space) ----
        late = ctx.enter_context(tc.tile_pool(name="late", bufs=1))
        wo_t = []
        for p in range(2):
            t = late.tile([128, DIM], F32R, name=f"wo{p}", tag=f"wo{p}")
            nc.sync.dma_start(t[:], woTp[p])
            wo_t.append(t)
        OTs = [
            late.tile([D + 1, N], F32, name=f"OTs{h}", tag=f"OTs{h}")
            for h in range(HPC)
        ]
        AT = [late.tile([128, N], F32R, name=f"AT{p}", tag=f"AT{p}") for p in range(2)]
        rbd = ctx.enter_context(tc.tile_pool(name="rbd", bufs=2))
        ysb = ctx.enter_context(tc.tile_pool(name="ysb", bufs=3))

        def norm_head(h, ih):
            # AT[pair] rows <- OTs_h[0:D] * broadcast(1/den_h), one i-half
            p, lo = h // 2, (h % 2) * D
            io = ih * 1024
            dtmp = rbd.tile([1, N // 2], F32, name=f"dtmp{h}_{ih}", tag="dtmp")
            nc.vector.tensor_copy(dtmp[:], OTs[h][D : D + 1, io : io + 1024])
            bden = rbd.tile([D, N // 2], F32, name=f"bden{h}_{ih}", tag="bden")
            nc.gpsimd.partition_broadcast(bden[:], dtmp[0:1, :], channels=D)
            rbb = rbd.tile([D, N // 2], F32, name=f"rbb{h}_{ih}", tag="rbb")
            nc.vector.reciprocal_approx_fast(rbb[:], bden[:])
            nc.vector.tensor_mul(
                AT[p][lo : lo + D, io : io + 1024],
                OTs[h][0:D, io : io + 1024],
                rbb[:],
            )

        # ---- attention ----
        for p in range(2):
            hA, hB = 2 * p, 2 * p + 1
            for ih in range(2):
                ioff = ih * 1024
                otA = pot.tile([D + 1, N // 2], F32, name=f"otA{p}_{ih}", tag="otA")
                otB = pot.tile([D + 1, N // 2], F32, name=f"otB{p}_{ih}", tag="otB")
                if p == 0 and ih == 0:
                    pending = list(prefill)
                else:
                    pending = []
                for jc in range(len(pending), NJC):
                    pending.append(att_S(p, ih, jc))
                    # flush PV matmuls in 16-mm bursts (keeps PE runs long)
                    if jc % 4 == 3:
                        for jcf, pA, pB in pending:
                            for ib in range(2):
                                nc.tensor.matmul(
                                    otA[:, ib * 512 : (ib + 1) * 512],
                                    VA[jcf][:, hA, :],
                                    pA[:, ib * 512 : (ib + 1) * 512],
                                    start=(jcf == 0),
                                    stop=(jcf == NJC - 1),
                                )
                            for ib in range(2):
                                nc.tensor.matmul(
                                    otB[:, ib * 512 : (ib + 1) * 512],
                                    VA[jcf][:, hB, :],
                                    pB[:, ib * 512 : (ib + 1) * 512],
                                    start=(jcf == 0),
                                    stop=(jcf == NJC - 1),
                                )
                        pending = []
                nc.vector.tensor_copy(OTs[hA][:, ioff : ioff + 1024], otA[:])
                nc.vector.tensor_copy(OTs[hB][:, ioff : ioff + 1024], otB[:])
                # normalize this i-half while the next block's attention runs
                norm_head(hA, ih)
                norm_head(hB, ih)

        # ---- output projection: Y[n,:] = sum_p AT_p[:,n]^T @ WoT_p (K=128) ----
        for nchunk in range(NJC):
            py = (ps2 if nchunk % 4 < 2 else pot).tile(
                [128, DIM], F32, name=f"py{nchunk}", tag=PTAGS[nchunk % 4],
                bufs=1,
            )
            for oh in range(2):
                for p in range(2):
                    nc.tensor.matmul(
                        py[:, oh * 512 : (oh + 1) * 512],
                        AT[p][:, nchunk * 128 : (nchunk + 1) * 128],
                        wo_t[p][:, oh * 512 : (oh + 1) * 512],
                        start=(p == 0),
                        stop=(p == 1),
                    )
            yt = ysb.tile([128, DIM], F32, name=f"yt{nchunk}", tag="yt")
            if nchunk % 2 == 0:
                nc.scalar.copy(yt[:], py[:])
            else:
                nc.vector.tensor_copy(yt[:], py[:])
            (nc.sync if nchunk % 2 == 0 else nc.scalar).dma_start(
                y[nchunk * 128 : (nchunk + 1) * 128, :], yt[:]
            )

    nc.compile()
    return nc


def _get_nc():
    if "nc" not in _CACHE:
        _CACHE["nc"] = _build_nc()
    return _CACHE["nc"]


def kernel(x, tar, mask, W_qv, W_k, W_out, b_out):
    from concourse import bass_utils

    x = np.asarray(x, np.float32)
    tar = np.asarray(tar, np.float32)
    mask = np.asarray(mask).astype(bool)
    W_qv = np.asarray(W_qv, np.float32)
    W_k = np.asarray(W_k, np.float32)
    W_out = np.asarray(W_out, np.float32)
    b_out = np.asarray(b_out, np.float32)

    m_pad = np.concatenate([np.ones((B, 1), bool), mask], axis=1)  # [B, N]
    maskadd_f = np.where(m_pad, 0.0, -30000.0).astype(np.float32)

    nc = _get_nc()
    in_maps = []
    for c in range(NCORES):
        b = c // 4
        h0 = (c % 4) * HPC
        rows = slice(h0 * D, h0 * D + HPC * D)
        vrows = slice(DIM + h0 * D, DIM + h0 * D + HPC * D)
        in_maps.append(
            {
                "xT": np.ascontiguousarray(x[b].T),
                "tarT": np.ascontiguousarray(tar[b].T),
                "wqT": np.ascontiguousarray((W_qv[rows] * np.float32(0.03125)).T),
                "wvT": np.ascontiguousarray(W_qv[vrows].T),
                "wkT": np.ascontiguousarray(W_k[rows].T),
                "woTp": np.ascontiguousarray(W_out[:, rows].T.reshape(2, 128, DIM)),
                "maskadd": np.ascontiguousarray(maskadd_f[b].reshape(NJC, 128).T),
                "onesc": np.ones((128, HPC), np.float32),
            }
        )

    res = bass_utils.run_bass_kernel_spmd(nc, in_maps, core_ids=list(range(NCORES)))
    out = np.empty((B, N, DIM), np.float32)
    for b in range(B):
        acc = res.results[4 * b]["y"].copy()
        for c in range(4 * b + 1, 4 * b + 4):
            acc += res.results[c]["y"]
        acc += b_out
        acc[~m_pad[b]] = np.nan
        out[b] = acc
    return out


# revision 31
# speedup vs baseline: 1.0108x; 1.0017x over previous
"""Trainium2 Bass kernel for nn_Attention_Dec (dense cross-attention, B=2 N=2048
DIM=1024 H=16 heads of d=64, padding mask, softmax over x-positions).

Sharding: core c handles batch b=c//4 and 4 heads h0=(c%4)*4 (tensor-parallel
over heads within a batch).  Each core computes a partial output
Y_c = (softmax-attn for its 4 heads) @ W_out[:, cols].T  of shape [2048, 1024];
host sums the 4 partials per batch, adds b_out, and writes NaN rows where the
(front-padded) mask is False — matching the reference's all--inf softmax NaNs.

Device-side per core (all matmuls float32r, fp32 PSUM accumulate):
  QT[d,j] = (Wq_h/32) @ x^T       (j = x positions; scale folded into weights)
  V[j,d]  = x @ Wv_h^T            (+ ones column for the softmax denominator)
  KT[d,i] = Wk_h @ tar^T          (i = tar positions)
  S^T[j,i] = QT^T·KT  (per head, K=64 contraction, head pairs row-tiled)
  P^T = exp(S^T + maskadd_j)      (per-partition ACT bias masks x positions;
                                   -30000 underflows exp to exactly 0)
  OT[d,i] = sum_j V_aug[j,d]·P^T[j,i]  (row 64 = softmax denominator den_i)
  AT[d,i] = OT[d,i] * (1/den_i)   (gpsimd partition_broadcast of the den row +
                                   DVE reciprocal_approx_fast + tensor_mul)
  Y[n,o] += AT_pair[:,n]^T @ WoT_pair  (K=128, accumulated over head pairs)

PSUM: 4 two-bank slots (sA, sB, otA, otB).  Attention keeps S^T ping-ponging
on sA/sB while otA/otB hold the two PV accumulators of the current head pair;
projections/final reuse the same slots.
"""

from contextlib import ExitStack

import numpy as np

B, N, DIM, H = 2, 2048, 1024, 16
D = 64
HPC = 4  # heads per core
NCORES = 8
NJC = N // 128  # 16 j-chunks
NKC = DIM // 128  # 8 contraction chunks

_CACHE = {}


def _build_nc():
    import concourse.tile as tile
    from concourse import bacc, mybir
    from concourse.tile import add_dep_helper

    F32 = mybir.dt.float32
    F32R = mybir.dt.float32r
    EXP = mybir.ActivationFunctionType.Exp

    nc = bacc.Bacc("TRN2", debug=False, num_devices=NCORES)

    xT = nc.dram_tensor("xT", [DIM, N], F32R, kind="ExternalInput").ap()
    tarT = nc.dram_tensor("tarT", [DIM, N], F32R, kind="ExternalInput").ap()
    wqT = nc.dram_tensor("wqT", [DIM, HPC * D], F32R, kind="ExternalInput").ap()
    wvT = nc.dram_tensor("wvT", [DIM, HPC * D], F32R, kind="ExternalInput").ap()
    wkT = nc.dram_tensor("wkT", [DIM, HPC * D], F32R, kind="ExternalInput").ap()
    woTp = nc.dram_tensor("woTp", [2, 128, DIM], F32R, kind="ExternalInput").ap()
    maskadd = nc.dram_tensor("maskadd", [128, NJC], F32, kind="ExternalInput").ap()
    onesc = nc.dram_tensor("onesc", [128, HPC], F32R, kind="ExternalInput").ap()
    y = nc.dram_tensor("y", [N, DIM], F32, kind="ExternalOutput").ap()

    with tile.TileContext(nc) as tc, ExitStack() as ctx:
        consts = ctx.enter_context(tc.tile_pool(name="consts", bufs=1))
        wpool = ctx.enter_context(tc.tile_pool(name="wpool", bufs=1))
        qkv = ctx.enter_context(tc.tile_pool(name="qkv", bufs=1))
        ps2 = ctx.enter_context(tc.tile_pool(name="ps2", bufs=1, space="PSUM"))
        pot = ctx.enter_context(tc.tile_pool(name="pot", bufs=1, space="PSUM"))

        mk = consts.tile([128, NJC], F32, name="mk")
        nc.scalar.dma_start(mk[:], maskadd[:])
        ones_c = consts.tile([128, HPC], F32R, name="ones_c")
        nc.scalar.dma_start(ones_c[:], onesc[:])

        wq_t, wv_t, wk_t = [], [], []

        KT = [qkv.tile([128, N], F32R, name=f"KT{p}", tag=f"KT{p}") for p in range(2)]
        QT = [qkv.tile([128, N], F32R, name=f"QT{p}", tag=f"QT{p}") for p in range(2)]
        VA_all = qkv.tile([128, NJC, HPC, D + 1], F32R, name="VA_all")
        VA = [VA_all[:, jc] for jc in range(NJC)]
        PTAGS = ["sA", "sB", "otA", "otB"]

        def proj_psum(i, nm):
            return (ps2 if i < 2 else pot).tile(
                [128, N // 2], F32, name=f"{nm}{i}", tag=PTAGS[i], bufs=1
            )

        ptp = ctx.enter_context(tc.tile_pool(name="ptp", bufs=10))

        def att_S(p, ih, jc):
            """S^T pair matmuls + masked exp for one j-chunk; returns PT pair."""
            ioff = ih * 1024
            stA = ps2.tile(
                [128, N // 2], F32, name=f"stA{p}_{jc}_{ih}", tag="sA", bufs=1
            )
            stB = ps2.tile(
                [128, N // 2], F32, name=f"stB{p}_{jc}_{ih}", tag="sB", bufs=1
            )
            for ib in range(2):
                nc.tensor.matmul(
                    stA[:, ib * 512 : (ib + 1) * 512],
                    QT[p][0:D, jc * 128 : (jc + 1) * 128],
                    KT[p][0:D, ioff + ib * 512 : ioff + (ib + 1) * 512],
                    start=True,
                    stop=True,
                )
            for ib in range(2):
                nc.tensor.matmul(
                    stB[:, ib * 512 : (ib + 1) * 512],
                    QT[p][D:128, jc * 128 : (jc + 1) * 128],
                    KT[p][D:128, ioff + ib * 512 : ioff + (ib + 1) * 512],
                    start=True,
                    stop=True,
                )
            ptA = ptp.tile([128, N // 2], F32R, name=f"ptA{p}_{jc}_{ih}", tag="pt")
            nc.scalar.activation(
                ptA[:], stA[:], EXP, bias=mk[:, jc : jc + 1], scale=1.0
            )
            ptB = ptp.tile([128, N // 2], F32R, name=f"ptB{p}_{jc}_{ih}", tag="pt")
            nc.scalar.activation(
                ptB[:], stB[:], EXP, bias=mk[:, jc : jc + 1], scale=1.0
            )
            return (jc, ptA, ptB)

        # ---- QT + V from xT ----
        with tc.tile_pool(name="xt", bufs=1) as xtp:
            xt = []
            for kc in range(NKC):
                t = xtp.tile([128, N], F32R, name=f"xt{kc}", tag=f"xt{kc}")
                (nc.sync if kc % 2 == 0 else nc.scalar).dma_start(
                    t[:], xT[kc * 128 : (kc + 1) * 128, :]
                )
                xt.append(t)
                # weights ride between the activation chunks (xt0 lands first)
                t = wpool.tile([128, HPC * D], F32R, name=f"wq{kc}", tag=f"wq{kc}")
                nc.sync.dma_start(t[:], wqT[kc * 128 : (kc + 1) * 128, :])
                wq_t.append(t)
                t = wpool.tile([128, HPC * D], F32R, name=f"wv{kc}", tag=f"wv{kc}")
                nc.scalar.dma_start(t[:], wvT[kc * 128 : (kc + 1) * 128, :])
                wv_t.append(t)
            # KT weights + tarT queue up behind xT on the DMA ring
            for kc in range(NKC):
                t = wpool.tile([128, HPC * D], F32R, name=f"wk{kc}", tag=f"wk{kc}")
                nc.scalar.dma_start(t[:], wkT[kc * 128 : (kc + 1) * 128, :])
                wk_t.append(t)

            psq = [proj_psum(i, "psq") for i in range(4)]
            for kc in range(NKC):
                for p in range(2):
                    for hf in range(2):
                        for ib in range(2):
                            off = hf * 1024 + ib * 512
                            nc.tensor.matmul(
                                psq[2 * p + hf][:, ib * 512 : (ib + 1) * 512],
                                wq_t[kc][:, p * 128 : (p + 1) * 128],
                                xt[kc][:, off : off + 512],
                                start=(kc == 0),
                                stop=(kc == NKC - 1),
                            )
            for p in range(2):
                for hf in range(2):
                    nc.vector.tensor_copy(
                        QT[p][:, hf * 1024 : (hf + 1) * 1024], psq[2 * p + hf][:]
                    )
            for jc in range(NJC):
                pv = (ps2 if jc % 2 == 0 else pot).tile(
                    [128, HPC * D], F32, name=f"pv{jc}",
                    tag=PTAGS[(jc % 2) * 2], bufs=1,
                )
                for kc in range(NKC):
                    nc.tensor.matmul(
                        pv[:],
                        xt[kc][:, jc * 128 : (jc + 1) * 128],
                        wv_t[kc][:],
                        start=(kc == 0),
                        stop=(kc == NKC - 1),
                    )
                nc.vector.tensor_copy(VA[jc][:, :, 0:D], pv[:])
                nc.vector.tensor_copy(VA[jc][:, :, D], ones_c[:])

        # ---- KT from tarT ----
        with tc.tile_pool(name="tart", bufs=1) as tartp:
            tt = []
            for kc in range(NKC):
                t = tartp.tile([128, N], F32R, name=f"tart{kc}", tag=f"tart{kc}")
                (nc.sync if kc % 2 == 0 else nc.scalar).dma_start(
                    t[:], tarT[kc * 128 : (kc + 1) * 128, :]
                )
                tt.append(t)
            psk = [proj_psum(i, "psk") for i in range(4)]
            for kc in range(NKC):
                for hf in range(2):
                    for ib in range(2):
                        off = hf * 1024 + ib * 512
                        nc.tensor.matmul(
                            psk[hf][:, ib * 512 : (ib + 1) * 512],
                            wk_t[kc][:, 0:128],
                            tt[kc][:, off : off + 512],
                            start=(kc == 0),
                            stop=(kc == NKC - 1),
                        )
            for hf in range(2):
                nc.vector.tensor_copy(
                    KT[0][:, hf * 1024 : (hf + 1) * 1024], psk[hf][:]
                )
            # attention pipeline prefill: S+exp for (p0,ih0) jc0; the p1 KT
            # matmuls below keep the PE dense while the first exps run
            prefill = [att_S(0, 0, 0)]
            for kc in range(NKC):
                for hf in range(2):
                    for ib in range(2):
                        off = hf * 1024 + ib * 512
                        nc.tensor.matmul(
                            psk[2 + hf][:, ib * 512 : (ib + 1) * 512],
                            wk_t[kc][:, 128:256],
                            tt[kc][:, off : off + 512],
                            start=(kc == 0),
                            stop=(kc == NKC - 1),
                        )
            for hf in range(2):
                nc.vector.tensor_copy(
                    KT[1][:, hf * 1024 : (hf + 1) * 1024], psk[2 + hf][:]
                )
            prefill.append(att_S(0, 0, 1))

        # ---- late pool (reuses released xt/tart space) ----
        late = ctx.enter_context(tc.tile_pool(name="late", bufs=1))
        wo_t = []
        for p in range(2):
            t = late.tile([128, DIM], F32R, name=f"wo{p}", tag=f"wo{p}")
            nc.sync.dma_start(t[:], woTp[p])
            wo_t.append(t)
        OTs = [
            late.tile([D + 1, N], F32, name=f"OTs{h}", tag=f"OTs{h}")
            for h in range(HPC)
        ]
        AT = [late.tile([128, N], F32R, name=f"AT{p}", tag=f"AT{p}") for p in range(2)]
        rbd = ctx.enter_context(tc.tile_pool(name="rbd", bufs=2))
        ysb = ctx.enter_context(tc.tile_pool(name="ysb", bufs=3))

        def norm_head(h, ih):
            # AT[pair] rows <- OTs_h[0:D] * broadcast(1/den_h), one i-half
            p, lo = h // 2, (h % 2) * D
            io = ih * 1024
            dtmp = rbd.tile([1, N // 2], F32, name=f"dtmp{h}_{ih}", tag="dtmp")
            nc.vector.tensor_copy(dtmp[:], OTs[h][D : D + 1, io : io + 1024])
            bden = rbd.tile([D, N // 2], F32, name=f"bden{h}_{ih}", tag="bden")
            nc.gpsimd.partition_broadcast(bden[:], dtmp[0:1, :], channels=D)
            rbb = rbd.tile([D, N // 2], F32, name=f"rbb{h}_{ih}", tag="rbb")
            nc.vector.reciprocal_approx_fast(rbb[:], bden[:])
            nc.vector.tensor_mul(
                AT[p][lo : lo + D, io : io + 1024],
                OTs[h][0:D, io : io + 1024],
                rbb[:],
            )

        # ---- attention ----
        for p in range(2):
            hA, hB = 2 * p, 2 * p + 1
            for ih in range(2):
                ioff = ih * 1024
                otA = pot.tile([D + 1, N // 2], F32, name=f"otA{p}_{ih}", tag="otA")
                otB = pot.tile([D + 1, N // 2], F32, name=f"otB{p}_{ih}", tag="otB")
                if p == 0 and ih == 0:
                    pending = list(prefill)
                else:
                    pending = []
                for jc in range(len(pending), NJC):
                    pending.append(att_S(p, ih, jc))
                    # flush PV matmuls in 16-mm bursts (keeps PE runs long)
                    if jc % 4 == 3:
                        for jcf, pA, pB in pending:
                            for ib in range(2):
                                nc.tensor.matmul(
                                    otA[:, ib * 512 : (ib + 1) * 512],
                                    VA[jcf][:, hA, :],
                                    pA[:, ib * 512 : (ib + 1) * 512],
                                    start=(jcf == 0),
                                    stop=(jcf == NJC - 1),
                                )
                            for ib in range(2):
                                nc.tensor.matmul(
                                    otB[:, ib * 512 : (ib + 1) * 512],
                                    VA[jcf][:, hB, :],
                                    pB[:, ib * 512 : (ib + 1) * 512],
                                    start=(jcf == 0),
                                    stop=(jcf == NJC - 1),
                                )
                        pending = []
                nc.vector.tensor_copy(OTs[hA][:, ioff : ioff + 1024], otA[:])
                nc.vector.tensor_copy(OTs[hB][:, ioff : ioff + 1024], otB[:])
                # normalize this i-half while the next block's attention runs
                norm_head(hA, ih)
                norm_head(hB, ih)

        # ---- output projection: Y[n,:] = sum_p AT_p[:,n]^T @ WoT_p (K=128) ----
        for nchunk in range(NJC):
            py = (ps2 if nchunk % 4 < 2 else pot).tile(
                [128, DIM], F32, name=f"py{nchunk}", tag=PTAGS[nchunk % 4],
                bufs=1,
            )
            for oh in range(2):
                for p in range(2):
                    nc.tensor.matmul(
                        py[:, oh * 512 : (oh + 1) * 512],
                        AT[p][:, nchunk * 128 : (nchunk + 1) * 128],
                        wo_t[p][:, oh * 512 : (oh + 1) * 512],
                        start=(p == 0),
                        stop=(p == 1),
                    )
            yt = ysb.tile([128, DIM], F32, name=f"yt{nchunk}", tag="yt")
            if nchunk % 2 == 0:
                nc.scalar.copy(yt[:], py[:])
            else:
                nc.vector.tensor_copy(yt[:], py[:])
            [nc.sync, nc.scalar, nc.gpsimd][nchunk % 3].dma_start(
                y[nchunk * 128 : (nchunk + 1) * 128, :], yt[:]
            )

    nc.compile()
    return nc


def _get_nc():
    if "nc" not in _CACHE:
        _CACHE["nc"] = _build_nc()
    return _CACHE["nc"]


def kernel(x, tar, mask, W_qv, W_k, W_out, b_out):
    from concourse import bass_utils

    x = np.asarray(x, np.float32)
    tar = np.asarray(tar, np.float32)
    mask = np.asarray(mask).astype(bool)
    W_qv = np.asarray(W_qv, np.float32)
    W_k = np.asarray(W_k, np.float32)
    W_out = np.asarray(W_out, np.float32)
    b_out = np.asarray(b_out, np.float32)

    m_pad = np.concatenate([np.ones((B, 1), bool), mask], axis=1)  # [B, N]
    maskadd_f = np.where(m_pad, 0.0, -30000.0).astype(np.float32)

    nc = _get_nc()
    in_maps = []
    for c in range(NCORES):
        b = c // 4
        h0 = (c % 4) * HPC
        rows = slice(h0 * D, h0 * D + HPC * D)
        vrows = slice(DIM + h0 * D, DIM + h0 * D + HPC * D)
        in_maps.append(
            {
                "xT": np.ascontiguousarray(x[b].T),
                "tarT": np.ascontiguousarray(tar[b].T),
                "wqT": np.ascontiguousarray((W_qv[rows] * np.float32(0.03125)).T),
                "wvT": np.ascontiguousarray(W_qv[vrows].T),
                "wkT": np.ascontiguousarray(W_k[rows].T),
                "woTp": np.ascontiguousarray(W_out[:, rows].T.reshape(2, 128, DIM)),
                "maskadd": np.ascontiguousarray(maskadd_f[b].reshape(NJC, 128).T),
                "onesc": np.ones((128, HPC), np.float32),
            }
        )

    res = bass_utils.run_bass_kernel_spmd(nc, in_maps, core_ids=list(range(NCORES)))
    out = np.empty((B, N, DIM), np.float32)
    for b in range(B):
        acc = res.results[4 * b]["y"].copy()
        for c in range(4 * b + 1, 4 * b + 4):
            acc += res.results[c]["y"]
        acc += b_out
        acc[~m_pad[b]] = np.nan
        out[b] = acc
    return out


# revision 32
# speedup vs baseline: 1.0331x; 1.0221x over previous
"""Trainium2 Bass kernel for nn_Attention_Dec (dense cross-attention, B=2 N=2048
DIM=1024 H=16 heads of d=64, padding mask, softmax over x-positions).

Sharding: core c handles batch b=c//4 and 4 heads h0=(c%4)*4 (tensor-parallel
over heads within a batch).  Each core computes a partial output
Y_c = (softmax-attn for its 4 heads) @ W_out[:, cols].T  of shape [2048, 1024];
host sums the 4 partials per batch, adds b_out, and writes NaN rows where the
(front-padded) mask is False — matching the reference's all--inf softmax NaNs.

Device-side per core (all matmuls float32r, fp32 PSUM accumulate):
  QT[d,j] = (Wq_h/32) @ x^T       (j = x positions; scale folded into weights)
  V[j,d]  = x @ Wv_h^T            (+ ones column for the softmax denominator)
  KT[d,i] = Wk_h @ tar^T          (i = tar positions)
  S^T[j,i] = QT^T·KT  (per head, K=64 contraction, head pairs row-tiled)
  P^T = exp(S^T + maskadd_j)      (per-partition ACT bias masks x positions;
                                   -30000 underflows exp to exactly 0)
  OT[d,i] = sum_j V_aug[j,d]·P^T[j,i]  (row 64 = softmax denominator den_i)
  AT[d,i] = OT[d,i] * (1/den_i)   (gpsimd partition_broadcast of the den row +
                                   DVE reciprocal_approx_fast + tensor_mul)
  Y[n,o] += AT_pair[:,n]^T @ WoT_pair  (K=128, accumulated over head pairs)

PSUM: 4 two-bank slots (sA, sB, otA, otB).  Attention keeps S^T ping-ponging
on sA/sB while otA/otB hold the two PV accumulators of the current head pair;
projections/final reuse the same slots.
"""

from contextlib import ExitStack

import numpy as np

B, N, DIM, H = 2, 2048, 1024, 16
D = 64
HPC = 4  # heads per core
NCORES = 8
NJC = N // 128  # 16 j-chunks
NKC = DIM // 128  # 8 contraction chunks

_CACHE = {}


def _build_nc():
    import concourse.tile as tile
    from concourse import bacc, mybir
    from concourse.tile import add_dep_helper

    F32 = mybir.dt.float32
    F32R = mybir.dt.float32r
    EXP = mybir.ActivationFunctionType.Exp

    nc = bacc.Bacc("TRN2", debug=False, num_devices=NCORES)

    xT = nc.dram_tensor("xT", [DIM, N], F32R, kind="ExternalInput").ap()
    tarT = nc.dram_tensor("tarT", [DIM, N], F32R, kind="ExternalInput").ap()
    wqT = nc.dram_tensor("wqT", [DIM, HPC * D], F32R, kind="ExternalInput").ap()
    wvT = nc.dram_tensor("wvT", [DIM, HPC * D], F32R, kind="ExternalInput").ap()
    wkT = nc.dram_tensor("wkT", [DIM, HPC * D], F32R, kind="ExternalInput").ap()
    woTp = nc.dram_tensor("woTp", [2, 128, DIM], F32R, kind="ExternalInput").ap()
    maskadd = nc.dram_tensor("maskadd", [128, NJC], F32, kind="ExternalInput").ap()
    onesc = nc.dram_tensor("onesc", [128, HPC], F32R, kind="ExternalInput").ap()
    y = nc.dram_tensor("y", [N, DIM], F32, kind="ExternalOutput").ap()

    with tile.TileContext(nc) as tc, ExitStack() as ctx:
        consts = ctx.enter_context(tc.tile_pool(name="consts", bufs=1))
        wpool = ctx.enter_context(tc.tile_pool(name="wpool", bufs=1))
        qkv = ctx.enter_context(tc.tile_pool(name="qkv", bufs=1))
        ps2 = ctx.enter_context(tc.tile_pool(name="ps2", bufs=1, space="PSUM"))
        pot = ctx.enter_context(tc.tile_pool(name="pot", bufs=1, space="PSUM"))

        mk = consts.tile([128, NJC], F32, name="mk")
        nc.scalar.dma_start(mk[:], maskadd[:])
        ones_c = consts.tile([128, HPC], F32R, name="ones_c")
        nc.scalar.dma_start(ones_c[:], onesc[:])

        wq_t, wv_t, wk_t = [], [], []

        KT = [qkv.tile([128, N], F32R, name=f"KT{p}", tag=f"KT{p}") for p in range(2)]
        QT = [qkv.tile([128, N], F32R, name=f"QT{p}", tag=f"QT{p}") for p in range(2)]
        VA_all = qkv.tile([128, NJC, HPC, D + 1], F32R, name="VA_all")
        VA = [VA_all[:, jc] for jc in range(NJC)]
        PTAGS = ["sA", "sB", "otA", "otB"]

        def proj_psum(i, nm):
            return (ps2 if i < 2 else pot).tile(
                [128, N // 2], F32, name=f"{nm}{i}", tag=PTAGS[i], bufs=1
            )

        ptp = ctx.enter_context(tc.tile_pool(name="ptp", bufs=10))

        def att_S(p, ih, jc):
            """S^T pair matmuls + masked exp for one j-chunk; returns PT pair."""
            ioff = ih * 1024
            stA = ps2.tile(
                [128, N // 2], F32, name=f"stA{p}_{jc}_{ih}", tag="sA", bufs=1
            )
            stB = ps2.tile(
                [128, N // 2], F32, name=f"stB{p}_{jc}_{ih}", tag="sB", bufs=1
            )
            for ib in range(2):
                nc.tensor.matmul(
                    stA[:, ib * 512 : (ib + 1) * 512],
                    QT[p][0:D, jc * 128 : (jc + 1) * 128],
                    KT[p][0:D, ioff + ib * 512 : ioff + (ib + 1) * 512],
                    start=True,
                    stop=True,
                )
            for ib in range(2):
                nc.tensor.matmul(
                    stB[:, ib * 512 : (ib + 1) * 512],
                    QT[p][D:128, jc * 128 : (jc + 1) * 128],
                    KT[p][D:128, ioff + ib * 512 : ioff + (ib + 1) * 512],
                    start=True,
                    stop=True,
                )
            ptA = ptp.tile([128, N // 2], F32R, name=f"ptA{p}_{jc}_{ih}", tag="pt")
            nc.scalar.activation(
                ptA[:], stA[:], EXP, bias=mk[:, jc : jc + 1], scale=1.0
            )
            ptB = ptp.tile([128, N // 2], F32R, name=f"ptB{p}_{jc}_{ih}", tag="pt")
            nc.scalar.activation(
                ptB[:], stB[:], EXP, bias=mk[:, jc : jc + 1], scale=1.0
            )
            return (jc, ptA, ptB)

        # ---- QT + V from xT ----
        with tc.tile_pool(name="xt", bufs=1) as xtp:
            xt = []
            for kc in range(NKC):
                t = xtp.tile([128, N], F32R, name=f"xt{kc}", tag=f"xt{kc}")
                (nc.sync if kc % 2 == 0 else nc.scalar).dma_start(
                    t[:], xT[kc * 128 : (kc + 1) * 128, :]
                )
                xt.append(t)
                # weights ride between the activation chunks (xt0 lands first)
                t = wpool.tile([128, HPC * D], F32R, name=f"wq{kc}", tag=f"wq{kc}")
                nc.sync.dma_start(t[:], wqT[kc * 128 : (kc + 1) * 128, :])
                wq_t.append(t)
                t = wpool.tile([128, HPC * D], F32R, name=f"wv{kc}", tag=f"wv{kc}")
                nc.scalar.dma_start(t[:], wvT[kc * 128 : (kc + 1) * 128, :])
                wv_t.append(t)
            # KT weights + tarT queue up behind xT on the DMA ring
            for kc in range(NKC):
                t = wpool.tile([128, HPC * D], F32R, name=f"wk{kc}", tag=f"wk{kc}")
                nc.scalar.dma_start(t[:], wkT[kc * 128 : (kc + 1) * 128, :])
                wk_t.append(t)

            psq = [proj_psum(i, "psq") for i in range(4)]
            for kc in range(NKC):
                for p in range(2):
                    for hf in range(2):
                        for ib in range(2):
                            off = hf * 1024 + ib * 512
                            nc.tensor.matmul(
                                psq[2 * p + hf][:, ib * 512 : (ib + 1) * 512],
                                wq_t[kc][:, p * 128 : (p + 1) * 128],
                                xt[kc][:, off : off + 512],
                                start=(kc == 0),
                                stop=(kc == NKC - 1),
                            )
            for p in range(2):
                for hf in range(2):
                    nc.vector.tensor_copy(
                        QT[p][:, hf * 1024 : (hf + 1) * 1024], psq[2 * p + hf][:]
                    )
            for jc in range(NJC):
                pv = (ps2 if jc % 2 == 0 else pot).tile(
                    [128, HPC * D], F32, name=f"pv{jc}",
                    tag=PTAGS[(jc % 2) * 2], bufs=1,
                )
                for kc in range(NKC):
                    nc.tensor.matmul(
                        pv[:],
                        xt[kc][:, jc * 128 : (jc + 1) * 128],
                        wv_t[kc][:],
                        start=(kc == 0),
                        stop=(kc == NKC - 1),
                    )
                nc.vector.tensor_copy(VA[jc][:, :, 0:D], pv[:])
                nc.vector.tensor_copy(VA[jc][:, :, D], ones_c[:])

        # ---- KT from tarT ----
        with tc.tile_pool(name="tart", bufs=1) as tartp:
            tt = []
            for kc in range(NKC):
                t = tartp.tile([128, N], F32R, name=f"tart{kc}", tag=f"tart{kc}")
                (nc.sync if kc % 2 == 0 else nc.scalar).dma_start(
                    t[:], tarT[kc * 128 : (kc + 1) * 128, :]
                )
                tt.append(t)
            psk = [proj_psum(i, "psk") for i in range(4)]
            for kc in range(NKC):
                for hf in range(2):
                    for ib in range(2):
                        off = hf * 1024 + ib * 512
                        nc.tensor.matmul(
                            psk[hf][:, ib * 512 : (ib + 1) * 512],
                            wk_t[kc][:, 0:128],
                            tt[kc][:, off : off + 512],
                            start=(kc == 0),
                            stop=(kc == NKC - 1),
                        )
            for hf in range(2):
                nc.vector.tensor_copy(
                    KT[0][:, hf * 1024 : (hf + 1) * 1024], psk[hf][:]
                )
            # attention pipeline prefill: S+exp for (p0,ih0) jc0; the p1 KT
            # matmuls below keep the PE dense while the first exps run
            prefill = [att_S(0, 0, 0)]
            for kc in range(NKC):
                for hf in range(2):
                    for ib in range(2):
                        off = hf * 1024 + ib * 512
                        nc.tensor.matmul(
                            psk[2 + hf][:, ib * 512 : (ib + 1) * 512],
                            wk_t[kc][:, 128:256],
                            tt[kc][:, off : off + 512],
                            start=(kc == 0),
                            stop=(kc == NKC - 1),
                        )
            for hf in range(2):
                nc.vector.tensor_copy(
                    KT[1][:, hf * 1024 : (hf + 1) * 1024], psk[2 + hf][:]
                )
            prefill.append(att_S(0, 0, 1))

        # ---- late pool (reuses released xt/tart space) ----
        late = ctx.enter_context(tc.tile_pool(name="late", bufs=1))
        wo_t = []
        for p in range(2):
            t = late.tile([128, DIM], F32R, name=f"wo{p}", tag=f"wo{p}")
            nc.sync.dma_start(t[:], woTp[p])
            wo_t.append(t)
        OTs = [
            late.tile([D + 1, N], F32, name=f"OTs{h}", tag=f"OTs{h}")
            for h in range(HPC)
        ]
        AT = [late.tile([128, N], F32R, name=f"AT{p}", tag=f"AT{p}") for p in range(2)]
        rbd = ctx.enter_context(tc.tile_pool(name="rbd", bufs=2))
        ysb = ctx.enter_context(tc.tile_pool(name="ysb", bufs=3))

        def norm_head(h, ih):
            # AT[pair] rows <- OTs_h[0:D] * broadcast(1/den_h), one i-half
            p, lo = h // 2, (h % 2) * D
            io = ih * 1024
            dtmp = rbd.tile([1, N // 2], F32, name=f"dtmp{h}_{ih}", tag="dtmp")
            nc.vector.tensor_copy(dtmp[:], OTs[h][D : D + 1, io : io + 1024])
            bden = rbd.tile([D, N // 2], F32, name=f"bden{h}_{ih}", tag="bden")
            nc.gpsimd.partition_broadcast(bden[:], dtmp[0:1, :], channels=D)
            rbb = rbd.tile([D, N // 2], F32, name=f"rbb{h}_{ih}", tag="rbb")
            nc.vector.reciprocal_approx_fast(rbb[:], bden[:])
            nc.vector.tensor_mul(
                AT[p][lo : lo + D, io : io + 1024],
                OTs[h][0:D, io : io + 1024],
                rbb[:],
            )

        # ---- attention ----
        for p in range(2):
            hA, hB = 2 * p, 2 * p + 1
            for ih in range(2):
                ioff = ih * 1024
                otA = pot.tile([D + 1, N // 2], F32, name=f"otA{p}_{ih}", tag="otA")
                otB = pot.tile([D + 1, N // 2], F32, name=f"otB{p}_{ih}", tag="otB")
                if p == 0 and ih == 0:
                    pending = list(prefill)
                else:
                    pending = []
                for jc in range(len(pending), NJC):
                    pending.append(att_S(p, ih, jc))
                    # flush PV matmuls in 16-mm bursts (keeps PE runs long)
                    if jc % 4 == 3:
                        for jcf, pA, pB in pending:
                            for ib in range(2):
                                nc.tensor.matmul(
                                    otA[:, ib * 512 : (ib + 1) * 512],
                                    VA[jcf][:, hA, :],
                                    pA[:, ib * 512 : (ib + 1) * 512],
                                    start=(jcf == 0),
                                    stop=(jcf == NJC - 1),
                                )
                            for ib in range(2):
                                nc.tensor.matmul(
                                    otB[:, ib * 512 : (ib + 1) * 512],
                                    VA[jcf][:, hB, :],
                                    pB[:, ib * 512 : (ib + 1) * 512],
                                    start=(jcf == 0),
                                    stop=(jcf == NJC - 1),
                                )
                        pending = []
                nc.vector.tensor_copy(OTs[hA][:, ioff : ioff + 1024], otA[:])
                nc.vector.tensor_copy(OTs[hB][:, ioff : ioff + 1024], otB[:])
                # normalize this i-half while the next block's attention runs
                norm_head(hA, ih)
                norm_head(hB, ih)

        # ---- output projection: Y[n,:] = sum_p AT_p[:,n]^T @ WoT_p (K=128) ----
        for nchunk in range(NJC):
            py = (ps2 if nchunk % 4 < 2 else pot).tile(
                [128, DIM], F32, name=f"py{nchunk}", tag=PTAGS[nchunk % 4],
                bufs=1,
            )
            for oh in range(2):
                for p in range(2):
                    nc.tensor.matmul(
                        py[:, oh * 512 : (oh + 1) * 512],
                        AT[p][:, nchunk * 128 : (nchunk + 1) * 128],
                        wo_t[p][:, oh * 512 : (oh + 1) * 512],
                        start=(p == 0),
                        stop=(p == 1),
                    )
            yt = ysb.tile([128, DIM], F32, name=f"yt{nchunk}", tag="yt")
            if nchunk % 2 == 0:
                nc.scalar.copy(yt[:], py[:])
            else:
                nc.vector.tensor_copy(yt[:], py[:])
            (nc.sync if nchunk % 2 == 0 else nc.scalar).dma_start(
                y[nchunk * 128 : (nchunk + 1) * 128, :], yt[:]
            )

    nc.compile()
    return nc


def _get_nc():
    if "nc" not in _CACHE:
        _CACHE["nc"] = _build_nc()
    return _CACHE["nc"]


def kernel(x, tar, mask, W_qv, W_k, W_out, b_out):
    from concourse import bass_utils

    x = np.asarray(x, np.float32)
    tar = np.asarray(tar, np.float32)
    mask = np.asarray(mask).astype(bool)
    W_qv = np.asarray(W_qv, np.float32)
    W_k = np.asarray(W_k, np.float32)
    W_out = np.asarray(W_out, np.float32)
    b_out = np.asarray(b_out, np.float32)

    m_pad = np.concatenate([np.ones((B, 1), bool), mask], axis=1)  # [B, N]
    maskadd_f = np.where(m_pad, 0.0, -30000.0).astype(np.float32)

    nc = _get_nc()
    in_maps = []
    for c in range(NCORES):
        b = c // 4
        h0 = (c % 4) * HPC
        rows = slice(h0 * D, h0 * D + HPC * D)
        vrows = slice(DIM + h0 * D, DIM + h0 * D + HPC * D)
        in_maps.append(
            {
                "xT": np.ascontiguousarray(x[b].T),
                "tarT": np.ascontiguousarray(tar[b].T),
                "wqT": np.ascontiguousarray((W_qv[rows] * np.float32(0.03125)).T),
                "wvT": np.ascontiguousarray(W_qv[vrows].T),
                "wkT": np.ascontiguousarray(W_k[rows].T),
                "woTp": np.ascontiguousarray(W_out[:, rows].T.reshape(2, 128, DIM)),
                "maskadd": np.ascontiguousarray(maskadd_f[b].reshape(NJC, 128).T),
                "onesc": np.ones((128, HPC), np.float32),
            }
        )

    res = bass_utils.run_bass_kernel_spmd(nc, in_maps, core_ids=list(range(NCORES)))
    out = np.empty((B, N, DIM), np.float32)
    for b in range(B):
        acc = res.results[4 * b]["y"].copy()
        for c in range(4 * b + 1, 4 * b + 4):
            acc += res.results[c]["y"]
        acc += b_out
        acc[~m_pad[b]] = np.nan
        out[b] = acc
    return out


# revision 33
# speedup vs baseline: 1.0622x; 1.0281x over previous
"""Trainium2 Bass kernel for nn_Attention_Dec (dense cross-attention, B=2 N=2048
DIM=1024 H=16 heads of d=64, padding mask, softmax over x-positions).

Sharding: core c handles batch b=c//4 and 4 heads h0=(c%4)*4 (tensor-parallel
over heads within a batch).  Each core computes a partial output
Y_c = (softmax-attn for its 4 heads) @ W_out[:, cols].T  of shape [2048, 1024];
host sums the 4 partials per batch, adds b_out, and writes NaN rows where the
(front-padded) mask is False — matching the reference's all--inf softmax NaNs.

Device-side per core (all matmuls float32r, fp32 PSUM accumulate):
  QT[d,j] = (Wq_h/32) @ x^T       (j = x positions; scale folded into weights)
  V[j,d]  = x @ Wv_h^T            (+ ones column for the softmax denominator)
  KT[d,i] = Wk_h @ tar^T          (i = tar positions)
  S^T[j,i] = QT^T·KT  (per head, K=64 contraction, head pairs row-tiled)
  P^T = exp(S^T + maskadd_j)      (per-partition ACT bias masks x positions;
                                   -30000 underflows exp to exactly 0)
  OT[d,i] = sum_j V_aug[j,d]·P^T[j,i]  (row 64 = softmax denominator den_i)
  AT[d,i] = OT[d,i] * (1/den_i)   (gpsimd partition_broadcast of the den row +
                                   DVE reciprocal_approx_fast + tensor_mul)
  Y[n,o] += AT_pair[:,n]^T @ WoT_pair  (K=128, accumulated over head pairs)

PSUM: 4 two-bank slots (sA, sB, otA, otB).  Attention keeps S^T ping-ponging
on sA/sB while otA/otB hold the two PV accumulators of the current head pair;
projections/final reuse the same slots.
"""

from contextlib import ExitStack

import numpy as np

B, N, DIM, H = 2, 2048, 1024, 16
D = 64
HPC = 4  # heads per core
NCORES = 8
NJC = N // 128  # 16 j-chunks
NKC = DIM // 128  # 8 contraction chunks

_CACHE = {}


def _build_nc():
    import concourse.tile as tile
    from concourse import bacc, mybir
    from concourse.tile import add_dep_helper

    F32 = mybir.dt.float32
    F32R = mybir.dt.float32r
    EXP = mybir.ActivationFunctionType.Exp

    nc = bacc.Bacc("TRN2", debug=False, num_devices=NCORES)

    xT = nc.dram_tensor("xT", [DIM, N], F32R, kind="ExternalInput").ap()
    tarT = nc.dram_tensor("tarT", [DIM, N], F32R, kind="ExternalInput").ap()
    wqT = nc.dram_tensor("wqT", [DIM, HPC * D], F32R, kind="ExternalInput").ap()
    wvT = nc.dram_tensor("wvT", [DIM, HPC * D], F32R, kind="ExternalInput").ap()
    wkT = nc.dram_tensor("wkT", [DIM, HPC * D], F32R, kind="ExternalInput").ap()
    woTp = nc.dram_tensor("woTp", [2, 128, DIM], F32R, kind="ExternalInput").ap()
    maskadd = nc.dram_tensor("maskadd", [128, NJC], F32, kind="ExternalInput").ap()
    onesc = nc.dram_tensor("onesc", [128, HPC], F32R, kind="ExternalInput").ap()
    y = nc.dram_tensor("y", [N, DIM], F32, kind="ExternalOutput").ap()

    with tile.TileContext(nc) as tc, ExitStack() as ctx:
        consts = ctx.enter_context(tc.tile_pool(name="consts", bufs=1))
        wpool = ctx.enter_context(tc.tile_pool(name="wpool", bufs=1))
        qkv = ctx.enter_context(tc.tile_pool(name="qkv", bufs=1))
        ps2 = ctx.enter_context(tc.tile_pool(name="ps2", bufs=1, space="PSUM"))
        pot = ctx.enter_context(tc.tile_pool(name="pot", bufs=1, space="PSUM"))

        mk = consts.tile([128, NJC], F32, name="mk")
        nc.scalar.dma_start(mk[:], maskadd[:])
        ones_c = consts.tile([128, HPC], F32R, name="ones_c")
        nc.scalar.dma_start(ones_c[:], onesc[:])

        wq_t, wv_t, wk_t = [], [], []

        KT = [qkv.tile([128, N], F32R, name=f"KT{p}", tag=f"KT{p}") for p in range(2)]
        QT = [qkv.tile([128, N], F32R, name=f"QT{p}", tag=f"QT{p}") for p in range(2)]
        VA_all = qkv.tile([128, NJC, HPC, D + 1], F32R, name="VA_all")
        VA = [VA_all[:, jc] for jc in range(NJC)]
        PTAGS = ["sA", "sB", "otA", "otB"]

        def proj_psum(i, nm):
            return (ps2 if i < 2 else pot).tile(
                [128, N // 2], F32, name=f"{nm}{i}", tag=PTAGS[i], bufs=1
            )

        ptp = ctx.enter_context(tc.tile_pool(name="ptp", bufs=10))

        def att_S(p, ih, jc):
            """S^T pair matmuls + masked exp for one j-chunk; returns PT pair."""
            ioff = ih * 1024
            stA = ps2.tile(
                [128, N // 2], F32, name=f"stA{p}_{jc}_{ih}", tag="sA", bufs=1
            )
            stB = ps2.tile(
                [128, N // 2], F32, name=f"stB{p}_{jc}_{ih}", tag="sB", bufs=1
            )
            for ib in range(2):
                nc.tensor.matmul(
                    stA[:, ib * 512 : (ib + 1) * 512],
                    QT[p][0:D, jc * 128 : (jc + 1) * 128],
                    KT[p][0:D, ioff + ib * 512 : ioff + (ib + 1) * 512],
                    start=True,
                    stop=True,
                )
            for ib in range(2):
                nc.tensor.matmul(
                    stB[:, ib * 512 : (ib + 1) * 512],
                    QT[p][D:128, jc * 128 : (jc + 1) * 128],
                    KT[p][D:128, ioff + ib * 512 : ioff + (ib + 1) * 512],
                    start=True,
                    stop=True,
                )
            ptA = ptp.tile([128, N // 2], F32R, name=f"ptA{p}_{jc}_{ih}", tag="pt")
            nc.scalar.activation(
                ptA[:], stA[:], EXP, bias=mk[:, jc : jc + 1], scale=1.0
            )
            ptB = ptp.tile([128, N // 2], F32R, name=f"ptB{p}_{jc}_{ih}", tag="pt")
            nc.scalar.activation(
                ptB[:], stB[:], EXP, bias=mk[:, jc : jc + 1], scale=1.0
            )
            return (jc, ptA, ptB)

        # ---- QT + V from xT ----
        with tc.tile_pool(name="xt", bufs=1) as xtp:
            xt = []
            for kc in range(NKC):
                t = xtp.tile([128, N], F32R, name=f"xt{kc}", tag=f"xt{kc}")
                (nc.sync if kc % 2 == 0 else nc.scalar).dma_start(
                    t[:], xT[kc * 128 : (kc + 1) * 128, :]
                )
                xt.append(t)
                # weights ride between the activation chunks (xt0 lands first)
                t = wpool.tile([128, HPC * D], F32R, name=f"wq{kc}", tag=f"wq{kc}")
                nc.sync.dma_start(t[:], wqT[kc * 128 : (kc + 1) * 128, :])
                wq_t.append(t)
                t = wpool.tile([128, HPC * D], F32R, name=f"wv{kc}", tag=f"wv{kc}")
                nc.scalar.dma_start(t[:], wvT[kc * 128 : (kc + 1) * 128, :])
                wv_t.append(t)
            # KT weights + tarT queue up behind xT on the DMA ring
            for kc in range(NKC):
                t = wpool.tile([128, HPC * D], F32R, name=f"wk{kc}", tag=f"wk{kc}")
                nc.scalar.dma_start(t[:], wkT[kc * 128 : (kc + 1) * 128, :])
                wk_t.append(t)

            psq = [proj_psum(i, "psq") for i in range(4)]
            for kc in range(NKC):
                for p in range(2):
                    for hf in range(2):
                        for ib in range(2):
                            off = hf * 1024 + ib * 512
                            nc.tensor.matmul(
                                psq[2 * p + hf][:, ib * 512 : (ib + 1) * 512],
                                wq_t[kc][:, p * 128 : (p + 1) * 128],
                                xt[kc][:, off : off + 512],
                                start=(kc == 0),
                                stop=(kc == NKC - 1),
                            )
            for p in range(2):
                for hf in range(2):
                    nc.vector.tensor_copy(
                        QT[p][:, hf * 1024 : (hf + 1) * 1024], psq[2 * p + hf][:]
                    )
            for jc in range(NJC):
                pv = (ps2 if jc % 2 == 0 else pot).tile(
                    [128, HPC * D], F32, name=f"pv{jc}",
                    tag=PTAGS[(jc % 2) * 2], bufs=1,
                )
                for kc in range(NKC):
                    nc.tensor.matmul(
                        pv[:],
                        xt[kc][:, jc * 128 : (jc + 1) * 128],
                        wv_t[kc][:],
                        start=(kc == 0),
                        stop=(kc == NKC - 1),
                    )
                nc.vector.tensor_copy(VA[jc][:, :, 0:D], pv[:])
                nc.vector.tensor_copy(VA[jc][:, :, D], ones_c[:])

        # ---- KT from tarT ----
        with tc.tile_pool(name="tart", bufs=1) as tartp:
            tt = []
            for kc in range(NKC):
                t = tartp.tile([128, N], F32R, name=f"tart{kc}", tag=f"tart{kc}")
                (nc.sync if kc % 2 == 0 else nc.scalar).dma_start(
                    t[:], tarT[kc * 128 : (kc + 1) * 128, :]
                )
                tt.append(t)
            psk = [proj_psum(i, "psk") for i in range(4)]
            for kc in range(NKC):
                for hf in range(2):
                    for ib in range(2):
                        off = hf * 1024 + ib * 512
                        nc.tensor.matmul(
                            psk[hf][:, ib * 512 : (ib + 1) * 512],
                            wk_t[kc][:, 0:128],
                            tt[kc][:, off : off + 512],
                            start=(kc == 0),
                            stop=(kc == NKC - 1),
                        )
            for hf in range(2):
                nc.vector.tensor_copy(
                    KT[0][:, hf * 1024 : (hf + 1) * 1024], psk[hf][:]
                )
            # attention pipeline prefill: S+exp for (p0,ih0) jc0; the p1 KT
            # matmuls below keep the PE dense while the first exps run
            prefill = [att_S(0, 0, 0)]
            for kc in range(NKC):
                for hf in range(2):
                    for ib in range(2):
                        off = hf * 1024 + ib * 512
                        nc.tensor.matmul(
                            psk[2 + hf][:, ib * 512 : (ib + 1) * 512],
                            wk_t[kc][:, 128:256],
                            tt[kc][:, off : off + 512],
                            start=(kc == 0),
                            stop=(kc == NKC - 1),
                        )
            for hf in range(2):
                nc.vector.tensor_copy(
                    KT[1][:, hf * 1024 : (hf + 1) * 1024], psk[2 + hf][:]
                )
            prefill.append(att_S(0, 0, 1))

        # ---- late pool (reuses released xt/tart space) ----
        late = ctx.enter_context(tc.tile_pool(name="late", bufs=1))
        wo_t = []
        for p in range(2):
            t = late.tile([128, DIM], F32R, name=f"wo{p}", tag=f"wo{p}")
            nc.sync.dma_start(t[:], woTp[p])
            wo_t.append(t)
        OTs = [
            late.tile([D + 1, N], F32, name=f"OTs{h}", tag=f"OTs{h}")
            for h in range(HPC)
        ]
        AT = [late.tile([128, N], F32R, name=f"AT{p}", tag=f"AT{p}") for p in range(2)]
        rbd = ctx.enter_context(tc.tile_pool(name="rbd", bufs=1))
        ysb = ctx.enter_context(tc.tile_pool(name="ysb", bufs=4))

        def norm_head(h, ih):
            # AT[pair] rows <- OTs_h[0:D] * broadcast(1/den_h), one i-half
            p, lo = h // 2, (h % 2) * D
            io = ih * 1024
            dtmp = rbd.tile([1, N // 2], F32, name=f"dtmp{h}_{ih}", tag="dtmp")
            nc.vector.tensor_copy(dtmp[:], OTs[h][D : D + 1, io : io + 1024])
            bden = rbd.tile([D, N // 2], F32, name=f"bden{h}_{ih}", tag="bden")
            nc.gpsimd.partition_broadcast(bden[:], dtmp[0:1, :], channels=D)
            rbb = rbd.tile([D, N // 2], F32, name=f"rbb{h}_{ih}", tag="rbb")
            nc.vector.reciprocal_approx_fast(rbb[:], bden[:])
            nc.vector.tensor_mul(
                AT[p][lo : lo + D, io : io + 1024],
                OTs[h][0:D, io : io + 1024],
                rbb[:],
            )

        # ---- attention ----
        for p in range(2):
            hA, hB = 2 * p, 2 * p + 1
            for ih in range(2):
                ioff = ih * 1024
                otA = pot.tile([D + 1, N // 2], F32, name=f"otA{p}_{ih}", tag="otA")
                otB = pot.tile([D + 1, N // 2], F32, name=f"otB{p}_{ih}", tag="otB")
                if p == 0 and ih == 0:
                    pending = list(prefill)
                else:
                    pending = []
                for jc in range(len(pending), NJC):
                    pending.append(att_S(p, ih, jc))
                    # flush PV matmuls in 16-mm bursts (keeps PE runs long)
                    if jc % 4 == 3:
                        for jcf, pA, pB in pending:
                            for ib in range(2):
                                nc.tensor.matmul(
                                    otA[:, ib * 512 : (ib + 1) * 512],
                                    VA[jcf][:, hA, :],
                                    pA[:, ib * 512 : (ib + 1) * 512],
                                    start=(jcf == 0),
                                    stop=(jcf == NJC - 1),
                                )
                            for ib in range(2):
                                nc.tensor.matmul(
                                    otB[:, ib * 512 : (ib + 1) * 512],
                                    VA[jcf][:, hB, :],
                                    pB[:, ib * 512 : (ib + 1) * 512],
                                    start=(jcf == 0),
                                    stop=(jcf == NJC - 1),
                                )
                        pending = []
                nc.vector.tensor_copy(OTs[hA][:, ioff : ioff + 1024], otA[:])
                nc.vector.tensor_copy(OTs[hB][:, ioff : ioff + 1024], otB[:])
                # normalize this i-half while the next block's attention runs
                norm_head(hA, ih)
                norm_head(hB, ih)

        # ---- output projection: Y[n,:] = sum_p AT_p[:,n]^T @ WoT_p (K=128) ----
        for nchunk in range(NJC):
            py = (ps2 if nchunk % 4 < 2 else pot).tile(
                [128, DIM], F32, name=f"py{nchunk}", tag=PTAGS[nchunk % 4],
                bufs=1,
            )
            for oh in range(2):
                for p in range(2):
                    nc.tensor.matmul(
                        py[:, oh * 512 : (oh + 1) * 512],
                        AT[p][:, nchunk * 128 : (nchunk + 1) * 128],
                        wo_t[p][:, oh * 512 : (oh + 1) * 512],
                        start=(p == 0),
                        stop=(p == 1),
                    )
            yt = ysb.tile([128, DIM], F32, name=f"yt{nchunk}", tag="yt")
            if nchunk % 2 == 0:
                nc.scalar.copy(yt[:], py[:])
            else:
                nc.vector.tensor_copy(yt[:], py[:])
            (nc.sync if nchunk % 2 == 0 else nc.scalar).dma_start(
                y[nchunk * 128 : (nchunk + 1) * 128, :], yt[:]
            )

    nc.compile()
    return nc


def _get_nc():
    if "nc" not in _CACHE:
        _CACHE["nc"] = _build_nc()
    return _CACHE["nc"]


def kernel(x, tar, mask, W_qv, W_k, W_out, b_out):
    from concourse import bass_utils

    x = np.asarray(x, np.float32)
    tar = np.asarray(tar, np.float32)
    mask = np.asarray(mask).astype(bool)
    W_qv = np.asarray(W_qv, np.float32)
    W_k = np.asarray(W_k, np.float32)
    W_out = np.asarray(W_out, np.float32)
    b_out = np.asarray(b_out, np.float32)

    m_pad = np.concatenate([np.ones((B, 1), bool), mask], axis=1)  # [B, N]
    maskadd_f = np.where(m_pad, 0.0, -30000.0).astype(np.float32)

    nc = _get_nc()
    in_maps = []
    for c in range(NCORES):
        b = c // 4
        h0 = (c % 4) * HPC
        rows = slice(h0 * D, h0 * D + HPC * D)
        vrows = slice(DIM + h0 * D, DIM + h0 * D + HPC * D)
        in_maps.append(
            {
                "xT": np.ascontiguousarray(x[b].T),
                "tarT": np.ascontiguousarray(tar[b].T),
                "wqT": np.ascontiguousarray((W_qv[rows] * np.float32(0.03125)).T),
                "wvT": np.ascontiguousarray(W_qv[vrows].T),
                "wkT": np.ascontiguousarray(W_k[rows].T),
                "woTp": np.ascontiguousarray(W_out[:, rows].T.reshape(2, 128, DIM)),
                "maskadd": np.ascontiguousarray(maskadd_f[b].reshape(NJC, 128).T),
                "onesc": np.ones((128, HPC), np.float32),
            }
        )

    res = bass_utils.run_bass_kernel_spmd(nc, in_maps, core_ids=list(range(NCORES)))
    out = np.empty((B, N, DIM), np.float32)
    for b in range(B):
        acc = res.results[4 * b]["y"].copy()
        for c in range(4 * b + 1, 4 * b + 4):
            acc += res.results[c]["y"]
        acc += b_out
        acc[~m_pad[b]] = np.nan
        out[b] = acc
    return out
